# revision 1
# baseline (speedup 1.0000x reference)
import numpy as np

import concourse.bass as bass
import concourse.tile as tile
from concourse import mybir
from concourse.masks import make_identity
from concourse.vector_clock import ScopedClock, VectorClock

B, S, J, QDIM, IDIM, HDIM = 64, 16, 256, 512, 512, 256
NCORES = 8
BLOC = B // NCORES
BS = BLOC * S
SCALE = 1.0 / np.sqrt(np.float32(HDIM))
NEG = -1.0e30

F32 = mybir.dt.float32
F32R = mybir.dt.float32r
U8 = mybir.dt.uint8
MULT = mybir.AluOpType.mult
ADD = mybir.AluOpType.add
AX = mybir.AxisListType.X
EXP = mybir.ActivationFunctionType.Exp


def _patched_drain_and_barrier(self, tick_clock, wait_clock):
    nc = self.nc
    gc = tick_clock.global_clock
    n = len(gc)
    procs = [p for p in range(n) if gc[p] > 0] or [0]
    for p in procs:
        vec = [0] * n
        vec[p] = gc[p]
        drain_inst = nc.sync.drain()
        wait_clock.add_sem_waits(drain_inst.ins, ScopedClock({None: VectorClock(vec)}))
    nc.all_engine_barrier()
    assert self.sems is not None
    popped = nc._tile_sem_poison_stack.pop()
    assert popped is self._sem_poison
    nc.clear_and_free_semaphores(list(self.sems.allocated().values()))
    nc.all_engine_barrier()


tile.TileContext._drain_and_barrier = _patched_drain_and_barrier

MAX_SYNC_WAITS = 1


def _split_sync_waits(nc, max_waits=MAX_SYNC_WAITS):
    plan = {}
    nop_names = set()
    for fn in nc.m.functions:
        for bb in fn.blocks:
            for inst in bb.instructions:
                si = getattr(inst, "sync_info", None)
                if si is None or not si.on_wait or len(si.on_wait) <= max_waits:
                    continue
                waits = list(si.on_wait)
                keep, extra = waits[:max_waits], waits[max_waits:]
                nops = []
                for i in range(0, len(extra), max_waits):
                    binst = nc.engines[inst.engine].drain(fusable=False)
                    nop = binst.ins
                    nop.sync_info = mybir.SyncInfo(
                        on_wait=extra[i:i + max_waits], on_update=[]
                    )
                    nops.append(nop)
                    nop_names.add(nop.name)
                inst.sync_info = mybir.SyncInfo(
                    on_wait=keep, on_update=list(si.on_update)
                )
                plan[inst.name] = nops
    if not plan:
        return 0
    for fn in nc.m.functions:
        for bb in fn.blocks:
            new_insts = []
            changed = False
            for inst in bb.instructions:
                if inst.name in nop_names:
                    changed = True
                    continue
                if inst.name in plan:
                    new_insts.extend(plan[inst.name])
                    changed = True
                new_insts.append(inst)
            if changed:
                bb.instructions = new_insts
    return len(plan)


def build_kernel(tc, x, qT, mask, wqT, wk, wvT, out):
    nc = tc.nc
    with (
        tc.tile_pool(name="singles", bufs=1) as singles,
        tc.tile_pool(name="xp", bufs=32) as xp,
        tc.tile_pool(name="prodp", bufs=4) as prodp,
        tc.tile_pool(name="pb", bufs=2) as pb,
        tc.tile_pool(name="ps_pooled", bufs=2, space="PSUM") as ps_pooled,
        tc.tile_pool(name="ps_misc", bufs=1, space="PSUM") as ps_misc,
        tc.tile_pool(name="ps_tr", bufs=2, space="PSUM") as ps_tr,
        tc.tile_pool(name="ps_wt", bufs=2, space="PSUM") as ps_wt,
        tc.tile_pool(name="ps_out", bufs=1, space="PSUM") as ps_out,
    ):
        ident = singles.tile([128, 128], F32)
        make_identity(nc, ident[:])

        wqT_sb = singles.tile([128, 4, HDIM], F32)
        nc.sync.dma_start(out=wqT_sb, in_=wqT.rearrange("(c p) h -> p c h", p=128))
        wk_sb = singles.tile([128, 2, IDIM], F32)
        nc.sync.dma_start(out=wk_sb, in_=wk.rearrange("(c p) i -> p c i", p=128))
        wvT_sb = singles.tile([128, 4, HDIM], F32)
        nc.sync.dma_start(out=wvT_sb, in_=wvT.rearrange("(c p) h -> p c h", p=128))
        qT_sb = singles.tile([128, 4, BLOC], F32)
        nc.sync.dma_start(out=qT_sb, in_=qT.rearrange("(c p) b -> p c b", p=128))


        QT_ps = ps_misc.tile([128, 2, BLOC], F32, tag="misc")
        for hc in range(2):
            for ic in range(4):
                nc.tensor.matmul(
                    QT_ps[:, hc, :],
                    lhsT=wqT_sb[:, ic, hc * 128:(hc + 1) * 128],
                    rhs=qT_sb[:, ic, :],
                    start=(ic == 0), stop=(ic == 3),
                )
        QT_sb = singles.tile([128, 2, BLOC], F32)
        nc.vector.tensor_copy(out=QT_sb, in_=QT_ps)

        qkT_ps = ps_misc.tile([128, 4, BLOC], F32, tag="misc")
        for ic in range(4):
            for hc in range(2):
                nc.tensor.matmul(
                    qkT_ps[:, ic, :],
                    lhsT=wk_sb[:, hc, ic * 128:(ic + 1) * 128],
                    rhs=QT_sb[:, hc, :],
                    start=(hc == 0), stop=(hc == 1),
                )
        qkT_sb = singles.tile([128, 4, BLOC], F32)
        nc.vector.tensor_copy(out=qkT_sb, in_=qkT_ps)

        qk_sb = singles.tile([BLOC, 4, 128], F32)
        for ic in range(4):
            qk_tr = ps_misc.tile([BLOC, 128], F32, tag="misc")
            nc.tensor.transpose(qk_tr, qkT_sb[:, ic, :], ident[:])
            nc.vector.tensor_copy(out=qk_sb[:, ic, :], in_=qk_tr)
        qk_dram = nc.dram_tensor("qk_scratch", [BLOC, IDIM], F32)
        nc.sync.dma_start(out=qk_dram[:], in_=qk_sb[:, :, :])
        qkb_all = singles.tile([128, BLOC, IDIM], F32)
        for b in range(BLOC):
            src = qk_dram[b:b + 1, :]
            src_bcast = bass.AP(
                tensor=src.tensor, offset=src.offset,
                ap=[[0, 128]] + list(src.ap[1:]),
            )
            nc.sync.dma_start(out=qkb_all[:, b, :], in_=src_bcast)

        pooled_sb = singles.tile([BS, IDIM], F32)

        for b in range(BLOC):
            xts = []
            scoresB = pb.tile([128, 2 * S], F32)
            mask_u8 = pb.tile([S, J], U8)
            nc.sync.dma_start(out=mask_u8, in_=mask[b * S:(b + 1) * S, :])
            mask_f = pb.tile([S, J], F32)
            nc.vector.tensor_copy(out=mask_f, in_=mask_u8)
            for s in range(S):
                xt = xp.tile([128, 2, IDIM], F32R)
                nc.sync.dma_start(
                    out=xt,
                    in_=x[b * S + s].rearrange("(jc p) i -> p jc i", p=128).bitcast(F32R),
                )
                xts.append(xt)
                for jc in range(2):
                    prod = prodp.tile([128, IDIM], F32)
                    nc.vector.scalar_tensor_tensor(
                        out=prod,
                        in0=xt[:, jc, :].bitcast(F32),
                        scalar=float(SCALE),
                        in1=qkb_all[:, b, :],
                        op0=MULT,
                        op1=MULT,
                        accum_out=scoresB[:, jc * S + s: jc * S + s + 1],
                    )

            scores_b = pb.tile([S, J], F32)
            for jc in range(2):
                tr = ps_tr.tile([S, 128], F32)
                nc.tensor.transpose(tr, scoresB[:, jc * S:(jc + 1) * S], ident[:])
                nc.vector.tensor_copy(out=scores_b[:, jc * 128:(jc + 1) * 128], in_=tr)

            sm = pb.tile([S, J], F32)
            nc.vector.scalar_tensor_tensor(
                out=sm, in0=mask_f, scalar=NEG,
                in1=scores_b, op0=MULT, op1=ADD,
            )
            mx = pb.tile([S, 1], F32)
            nc.vector.reduce_max(out=mx, in_=sm, axis=AX)
            negm = pb.tile([S, 1], F32)
            nc.vector.tensor_scalar_mul(out=negm, in0=mx, scalar1=-1.0)
            wexp = pb.tile([S, J], F32)
            sumex = pb.tile([S, 1], F32)
            nc.scalar.activation(
                out=wexp, in_=sm, func=EXP, bias=negm, scale=1.0, accum_out=sumex
            )
            rinv = pb.tile([S, 1], F32)
            nc.vector.reciprocal(out=rinv, in_=sumex)
            wts = pb.tile([S, J], F32)
            nc.vector.tensor_scalar_mul(out=wts, in0=wexp, scalar1=rinv)

            wt_sb = pb.tile([128, 2 * S], F32)
            for jc in range(2):
                wtr = ps_wt.tile([128, S], F32)
                nc.tensor.transpose(wtr, wts[:, jc * 128:(jc + 1) * 128], ident[:S, :S])
                nc.vector.tensor_copy(out=wt_sb[:, jc * S:(jc + 1) * S], in_=wtr)

            wtblk_f = pb.tile([128, 2 * S * S], F32)
            nc.vector.memset(wtblk_f, 0.0)
            for jc in range(2):
                nc.vector.tensor_copy(
                    out=wtblk_f[:, jc * S: 2 * S * S: 2 * S + 1],
                    in_=wt_sb[:, jc * S:(jc + 1) * S],
                )
            wtblk = pb.tile([128, 2 * S * S], F32R)
            nc.sync.dma_start(out=wtblk, in_=wtblk_f.bitcast(F32R))
            wtblk3 = wtblk.rearrange("p (k s) -> p k s", s=S)
            pooled_b = ps_pooled.tile([S, IDIM], F32)
            for k in range(2 * S):
                s, jc = k // 2, k % 2
                nc.tensor.matmul(
                    pooled_b,
                    lhsT=wtblk3[:, k, :],
                    rhs=xts[s][:, jc, :],
                    start=(k == 0), stop=(k == 2 * S - 1),
                )
            pooled_tmp = pb.tile([S, IDIM], F32)
            nc.vector.tensor_copy(out=pooled_tmp, in_=pooled_b)
            nc.sync.dma_start(out=pooled_sb[b * S:(b + 1) * S, :], in_=pooled_tmp)

        pooledT_sb = singles.tile([128, 4, BS], F32)
        for ic in range(4):
            ptr = ps_misc.tile([128, BS], F32, tag="misc")
            nc.tensor.transpose(ptr, pooled_sb[:, ic * 128:(ic + 1) * 128], ident[:])
            nc.vector.tensor_copy(out=pooledT_sb[:, ic, :], in_=ptr)
        out_ps = ps_out.tile([BS, HDIM], F32)
        for ic in range(4):
            nc.tensor.matmul(
                out_ps,
                lhsT=pooledT_sb[:, ic, :],
                rhs=wvT_sb[:, ic, :],
                start=(ic == 0), stop=(ic == 3),
            )
        out_sb = singles.tile([BS, HDIM], F32)
        nc.vector.tensor_copy(out=out_sb, in_=out_ps)
        nc.sync.dma_start(out=out[:], in_=out_sb)


def build_bass():
    nc = bass.Bass("TRN2", target_bir_lowering=False, debug=False)
    x = nc.dram_tensor("x", [BS, J, IDIM], F32, kind="ExternalInput")
    qT = nc.dram_tensor("qT", [IDIM, BLOC], F32, kind="ExternalInput")
    mask = nc.dram_tensor("mask", [BS, J], U8, kind="ExternalInput")
    wqT = nc.dram_tensor("wqT", [IDIM, HDIM], F32, kind="ExternalInput")
    wk = nc.dram_tensor("wk", [HDIM, IDIM], F32, kind="ExternalInput")
    wvT = nc.dram_tensor("wvT", [IDIM, HDIM], F32, kind="ExternalInput")
    out = nc.dram_tensor("out", [BS, HDIM], F32, kind="ExternalOutput")
    with tile.TileContext(nc) as tc:
        build_kernel(tc, x, qT, mask, wqT, wk, wvT, out)
    _split_sync_waits(nc)
    return nc


def make_in_maps(query, other_semesters, mask, Wq, Wk, Wv):
    wqT = np.ascontiguousarray(Wq.T)
    wvT = np.ascontiguousarray(Wv.T)
    wk = np.ascontiguousarray(Wk)
    in_maps = []
    for c in range(NCORES):
        b0 = c * BLOC
        in_maps.append({
            "x": np.ascontiguousarray(
                other_semesters[b0:b0 + BLOC].reshape(BS, J, IDIM)
            ),
            "qT": np.ascontiguousarray(query[b0:b0 + BLOC].T),
            "mask": np.ascontiguousarray(
                mask[b0:b0 + BLOC].reshape(BS, J).view(np.uint8)
            ),
            "wqT": wqT,
            "wk": wk,
            "wvT": wvT,
        })
    return in_maps


_NC_CACHE = None


def get_nc():
    global _NC_CACHE
    if _NC_CACHE is None:
        _NC_CACHE = build_bass()
    return _NC_CACHE


def kernel(query, other_semesters, mask, Wq, Wk, Wv):
    from concourse.bass_utils import run_bass_kernel_spmd

    nc = get_nc()
    in_maps = make_in_maps(query, other_semesters, mask, Wq, Wk, Wv)
    res = run_bass_kernel_spmd(nc, in_maps, list(range(NCORES)), trace=False)
    out = np.empty((B, S, HDIM), dtype=np.float32)
    for c in range(NCORES):
        out[c * BLOC:(c + 1) * BLOC] = res.results[c]["out"].reshape(BLOC, S, HDIM)
    return out



# revision 32
# speedup vs baseline: 22.4154x; 22.4154x over previous
import numpy as np

import concourse.bass as bass
import concourse.tile as tile
from concourse import mybir
from concourse.masks import make_identity
from concourse.vector_clock import ScopedClock, VectorClock

B, S, J = 64, 16, 256
QDIM, IDIM, HDIM = 512, 512, 256
NCORES = 8
BLOC = B // NCORES
BS = BLOC * S
NCHUNK = 4
SCHUNK = S // NCHUNK
SCALE = 1.0 / np.sqrt(np.float32(HDIM))
NEG = -1.0e30

F32 = mybir.dt.float32
F32R = mybir.dt.float32r
U8 = mybir.dt.uint8
MULT = mybir.AluOpType.mult
ADD = mybir.AluOpType.add
AX = mybir.AxisListType.X
EXP = mybir.ActivationFunctionType.Exp
COPY = mybir.ActivationFunctionType.Copy


def _patched_drain_and_barrier(self, tick_clock, wait_clock):
    nc = self.nc
    gc = tick_clock.global_clock
    n = len(gc)
    procs = [p for p in range(n) if gc[p] > 0] or [0]
    for p in procs:
        vec = [0] * n
        vec[p] = gc[p]
        drain_inst = nc.sync.drain()
        wait_clock.add_sem_waits(drain_inst.ins, ScopedClock({None: VectorClock(vec)}))
    nc.all_engine_barrier()
    assert self.sems is not None
    popped = nc._tile_sem_poison_stack.pop()
    assert popped is self._sem_poison
    nc.clear_and_free_semaphores(list(self.sems.allocated().values()))
    nc.all_engine_barrier()


tile.TileContext._drain_and_barrier = _patched_drain_and_barrier

MAX_SYNC_WAITS = 1


def _split_sync_waits(nc, max_waits=MAX_SYNC_WAITS):
    plan = {}
    nop_names = set()
    for fn in nc.m.functions:
        for bb in fn.blocks:
            for inst in bb.instructions:
                si = getattr(inst, "sync_info", None)
                if si is None or not si.on_wait or len(si.on_wait) <= max_waits:
                    continue
                waits = list(si.on_wait)
                keep, extra = waits[:max_waits], waits[max_waits:]
                nops = []
                for i in range(0, len(extra), max_waits):
                    binst = nc.engines[inst.engine].drain(fusable=False)
                    nop = binst.ins
                    nop.sync_info = mybir.SyncInfo(
                        on_wait=extra[i:i + max_waits], on_update=[]
                    )
                    nops.append(nop)
                    nop_names.add(nop.name)
                inst.sync_info = mybir.SyncInfo(
                    on_wait=keep, on_update=list(si.on_update)
                )
                plan[inst.name] = nops
    if not plan:
        return 0
    for fn in nc.m.functions:
        for bb in fn.blocks:
            new_insts = []
            changed = False
            for inst in bb.instructions:
                if inst.name in nop_names:
                    changed = True
                    continue
                if inst.name in plan:
                    new_insts.extend(plan[inst.name])
                    changed = True
                new_insts.append(inst)
            if changed:
                bb.instructions = new_insts
    return len(plan)


def build_kernel(tc, x, qT, maskT, wqT, wk, wvT, out):
    nc = tc.nc
    with (
        tc.tile_pool(name="singles", bufs=1) as singles,
        tc.tile_pool(name="xp", bufs=2 * S + 2) as xp,
        tc.tile_pool(name="pb", bufs=2) as pb,
        tc.tile_pool(name="ps_pooled", bufs=2, space="PSUM") as ps_pooled,
        tc.tile_pool(name="ps_z", bufs=2, space="PSUM") as ps_z,
        tc.tile_pool(name="ps_ptr", bufs=2, space="PSUM") as ps_ptr,
        tc.tile_pool(name="ps_misc", bufs=1, space="PSUM") as ps_misc,
        tc.tile_pool(name="ps_out", bufs=1, space="PSUM") as ps_out,
    ):
        ident = singles.tile([128, 128], F32)
        make_identity(nc, ident[:])

        wqT_sb = singles.tile([128, 4, HDIM], F32)
        nc.scalar.dma_start(out=wqT_sb, in_=wqT.rearrange("(c p) h -> p c h", p=128))
        wk_sb = singles.tile([128, 2, IDIM], F32)
        nc.scalar.dma_start(out=wk_sb, in_=wk.rearrange("(c p) i -> p c i", p=128))
        wvT_sb = singles.tile([128, 4, HDIM], F32)
        nc.scalar.dma_start(out=wvT_sb, in_=wvT.rearrange("(c p) h -> p c h", p=128))
        qT_sb = singles.tile([128, 4, BLOC], F32)
        nc.scalar.dma_start(out=qT_sb, in_=qT.rearrange("(c p) b -> p c b", p=128))

        maskT_u8 = singles.tile([128, 2, BS], U8)
        nc.scalar.dma_start(out=maskT_u8, in_=maskT.rearrange("(jc p) r -> p jc r", p=128))
        maskT_neg = singles.tile([128, 2, BS], F32)
        nc.scalar.activation(out=maskT_neg, in_=maskT_u8, func=COPY,
                             scale=NEG)

        xts = []
        for b in range(BLOC):
            row = []
            for s in range(S):
                xt = xp.tile([128, 2, IDIM], F32R,
                             name=f"xt{b}_{s}", tag="xt")
                src = x[b * S + s]
                nc.sync.dma_start(
                    out=xt,
                    in_=src.rearrange("(jc p) i -> p jc i", p=128).bitcast(F32R),
                )
                row.append(xt)
            xts.append(row)

        QT_ps = ps_misc.tile([128, 2, BLOC], F32, tag="misc")
        for hc in range(2):
            for ic in range(4):
                nc.tensor.matmul(
                    QT_ps[:, hc, :],
                    lhsT=wqT_sb[:, ic, hc * 128:(hc + 1) * 128],
                    rhs=qT_sb[:, ic, :],
                    start=(ic == 0), stop=(ic == 3),
                )
        QT_sb = singles.tile([128, 2, BLOC], F32)
        nc.vector.tensor_copy(out=QT_sb, in_=QT_ps)

        qkT_ps = ps_misc.tile([128, 4, BLOC], F32, tag="misc")
        for ic in range(4):
            for hc in range(2):
                nc.tensor.matmul(
                    qkT_ps[:, ic, :],
                    lhsT=wk_sb[:, hc, ic * 128:(ic + 1) * 128],
                    rhs=QT_sb[:, hc, :],
                    start=(hc == 0), stop=(hc == 1),
                )
        qkT_sb = singles.tile([128, 4, BLOC], F32)
        nc.vector.tensor_copy(out=qkT_sb, in_=qkT_ps)

        qk_sb = singles.tile([BLOC, 4, 128], F32)
        for ic in range(4):
            qk_tr = ps_misc.tile([BLOC, 128], F32, tag="misc")
            nc.tensor.transpose(qk_tr, qkT_sb[:, ic, :], ident[:])
            nc.vector.tensor_copy(out=qk_sb[:, ic, :], in_=qk_tr)
        qk_dram = nc.dram_tensor("qk_scratch", [BLOC, IDIM], F32)
        nc.scalar.dma_start(out=qk_dram[:], in_=qk_sb[:, :, :])
        qkb_all = singles.tile([128, BLOC, IDIM], F32)
        for b in range(BLOC):
            src = qk_dram[b:b + 1, :]
            src_bcast = bass.AP(
                tensor=src.tensor, offset=src.offset,
                ap=[[0, 128]] + list(src.ap[1:]),
            )
            nc.scalar.dma_start(out=qkb_all[:, b, :], in_=src_bcast)

        wtblk = [singles.tile([128, 2 * S * S], F32, name=f"wtblk{i}")
                 for i in range(2)]
        wtblk_r = [singles.tile([128, 2 * S * S], F32R, name=f"wtblkr{i}")
                   for i in range(2)]
        nc.vector.memset(wtblk[0], 0.0)
        nc.vector.memset(wtblk[1], 0.0)

        scr_dve = singles.tile([128, IDIM], F32)

        ones_f = singles.tile([128, 2], F32)
        nc.vector.memset(ones_f, 1.0)
        ones_r = singles.tile([128, 2], F32R)
        nc.scalar.dma_start(out=ones_r, in_=ones_f.bitcast(F32R))

        state = {}

        def tail_recip(pb_idx):
            st = state[pb_idx]
            rinv = pb.tile([S, 1], F32, name=f"ri{pb_idx}", tag="ri")
            nc.vector.reciprocal(out=rinv, in_=st["z_ps"][:, 0:1])
            pooled_sc = pb.tile([S, IDIM], F32, name=f"psc{pb_idx}", tag="psc")
            nc.scalar.activation(
                out=pooled_sc, in_=st["pooled_ps"], func=COPY, scale=rinv,
            )
            st["pooled_sc"] = pooled_sc

        def tail_proj(pb_idx):
            st = state.pop(pb_idx)
            pooled_sc = st["pooled_sc"]
            pooledT = pb.tile([128, 4, S], F32, name=f"pT{pb_idx}", tag="pT")
            for ic in range(4):
                ptr = ps_ptr.tile([128, S], F32)
                nc.tensor.transpose(
                    ptr, pooled_sc[:, ic * 128:(ic + 1) * 128], ident[:S, :S]
                )
                nc.vector.tensor_copy(out=pooledT[:, ic, :], in_=ptr)
            out_ps = ps_out.tile([S, HDIM], F32)
            for ic in range(4):
                nc.tensor.matmul(
                    out_ps,
                    lhsT=pooledT[:, ic, :],
                    rhs=wvT_sb[:, ic, :],
                    start=(ic == 0), stop=(ic == 3),
                )
            out_sb = pb.tile([S, HDIM], F32, name=f"osb{pb_idx}", tag="osb")
            nc.scalar.activation(out=out_sb, in_=out_ps, func=COPY, scale=1.0)
            nc.scalar.dma_start(
                out=out[pb_idx * S:(pb_idx + 1) * S, :], in_=out_sb
            )

        for b in range(BLOC):
            sc = [pb.tile([128, S], F32, name=f"sc{b}_{i}", tag=f"sc_{i}")
                  for i in range(2)]
            msc = pb.tile([128, 2, S], F32, name=f"msc{b}", tag="msc")
            blk = wtblk[b % 2]
            blk_r = wtblk_r[b % 2]
            for ci in range(NCHUNK):
                for jc in range(2):
                    for t in range(SCHUNK):
                        s = ci * SCHUNK + t
                        nc.vector.scalar_tensor_tensor(
                            out=scr_dve,
                            in0=xts[b][ci * SCHUNK + t][:, jc, :].bitcast(F32),
                            scalar=float(SCALE),
                            in1=qkb_all[:, b, :],
                            op0=MULT,
                            op1=MULT,
                            accum_out=sc[jc][:, s:s + 1],
                        )
                for jc in range(2):
                    nc.gpsimd.tensor_add(
                        out=msc[:, jc, ci * SCHUNK:(ci + 1) * SCHUNK],
                        in0=maskT_neg[:, jc,
                                      b * S + ci * SCHUNK:
                                      b * S + (ci + 1) * SCHUNK],
                        in1=sc[jc][:, ci * SCHUNK:(ci + 1) * SCHUNK],
                    )
                for jc in range(2):
                    st0 = 132 * ci + 16 * jc
                    nc.scalar.activation(
                        out=blk[:, st0: st0 + 33 * (SCHUNK - 1) + 1: 33],
                        in_=msc[:, jc, ci * SCHUNK:(ci + 1) * SCHUNK],
                        func=EXP, scale=1.0,
                    )
                nc.scalar.dma_start(
                    out=blk_r[:, 128 * ci: 128 * (ci + 1)],
                    in_=blk[:, 128 * ci: 128 * (ci + 1)].bitcast(F32R),
                )
                if b > 0 and ci == 0:
                    tail_recip(b - 1)

            blk3 = blk_r.rearrange("p (k s) -> p k s", s=S)
            pooled_ps = ps_pooled.tile([S, IDIM], F32, name=f"pps{b}",
                                       tag="pps")
            z_ps = ps_z.tile([S, 2], F32, name=f"zps{b}", tag="zps")
            for n in range(2 * S):
                ci, t, jc = n // 8, (n // 2) % SCHUNK, n % 2
                s = ci * SCHUNK + t
                nc.tensor.matmul(
                    pooled_ps,
                    lhsT=blk3[:, 2 * s + jc, :],
                    rhs=xts[b][s][:, jc, :],
                    start=(n == 0), stop=(n == 2 * S - 1),
                )
                nc.tensor.matmul(
                    z_ps,
                    lhsT=blk3[:, 2 * s + jc, :],
                    rhs=ones_r,
                    start=(n == 0), stop=(n == 2 * S - 1),
                )
            state[b] = {"pooled_ps": pooled_ps, "z_ps": z_ps}
            if b > 0:
                tail_proj(b - 1)

        tail_recip(BLOC - 1)
        tail_proj(BLOC - 1)


def build_bass():
    nc = bass.Bass("TRN2", target_bir_lowering=False, debug=False)
    x = nc.dram_tensor("x", [BS, J, IDIM], F32, kind="ExternalInput")
    qT = nc.dram_tensor("qT", [IDIM, BLOC], F32, kind="ExternalInput")
    maskT = nc.dram_tensor("maskT", [J, BS], U8, kind="ExternalInput")
    wqT = nc.dram_tensor("wqT", [IDIM, HDIM], F32, kind="ExternalInput")
    wk = nc.dram_tensor("wk", [HDIM, IDIM], F32, kind="ExternalInput")
    wvT = nc.dram_tensor("wvT", [IDIM, HDIM], F32, kind="ExternalInput")
    out = nc.dram_tensor("out", [BS, HDIM], F32, kind="ExternalOutput")
    with tile.TileContext(nc) as tc:
        build_kernel(tc, x, qT, maskT, wqT, wk, wvT, out)
    _split_sync_waits(nc)
    return nc


def make_in_maps(query, other_semesters, mask, Wq, Wk, Wv):
    wqT = np.ascontiguousarray(Wq.T)
    wvT = np.ascontiguousarray(Wv.T)
    wk = np.ascontiguousarray(Wk)
    in_maps = []
    for c in range(NCORES):
        b0 = c * BLOC
        in_maps.append({
            "x": np.ascontiguousarray(
                other_semesters[b0:b0 + BLOC].reshape(BS, J, IDIM)
            ),
            "qT": np.ascontiguousarray(query[b0:b0 + BLOC].T),
            "maskT": np.ascontiguousarray(
                mask[b0:b0 + BLOC].reshape(BS, J).view(np.uint8).T
            ),
            "wqT": wqT,
            "wk": wk,
            "wvT": wvT,
        })
    return in_maps


_NC_CACHE = None


def get_nc():
    global _NC_CACHE
    if _NC_CACHE is None:
        _NC_CACHE = build_bass()
    return _NC_CACHE


def kernel(query, other_semesters, mask, Wq, Wk, Wv):
    from concourse.bass_utils import run_bass_kernel_spmd

    nc = get_nc()
    in_maps = make_in_maps(query, other_semesters, mask, Wq, Wk, Wv)
    res = run_bass_kernel_spmd(nc, in_maps, list(range(NCORES)), trace=False)
    out = np.empty((B, S, HDIM), dtype=np.float32)
    for c in range(NCORES):
        out[c * BLOC:(c + 1) * BLOC] = res.results[c]["out"].reshape(BLOC, S, HDIM)
    return out
